# revision 42
# baseline (speedup 1.0000x reference)
"""GraphSAGE (2-layer, DGL SAGEConv-mean) Trainium2 kernel — y-scheme.

Data-parallel over B (4 samples per core, 8 cores). Per (b,c) pair, with
A=adj, deg=max(indeg,1), D=diag(deg):

  y  = A^T x                      (level Y, 24 cols/pair)
  [R1'|R4|R5] = y @ [A00|B01|C01] (PE transpose of y + small matmuls,
                                   output lands node-major directly)
  t  = A^T (D^{-1} R5)            (level T2)
  w  = A^T R5                     (level W)
  OUT0 = dinv4*(t + R4) + (4*x@A00 + biasN)        [host-folded mb0]
  OUT1 = dinv*(A^T (R4 + D^{-1} w)) + R1' + biasN  (level A1)

vs the previous 6-level scheme this applies A^T to 4 slabs per pair
instead of 6 (96 vs 144 moving cols/pair). adj is stored fp8_e4m3
(exact for 0/1), halving its SBUF/DMA footprint. Small-weight products
use lhsT = y^T chunks so results come out node-major (no back-transpose).
"""
import sys

sys.path.insert(0, "/opt/trn_rl_repo")

import numpy as np
import ml_dtypes

from concourse import bass, bacc, tile, mybir
from concourse.bass_utils import run_bass_kernel_spmd

BF16 = mybir.dt.bfloat16
F32 = mybir.dt.float32
FP8 = mybir.dt.float8e4

N = 2048
L = 24
B = 32
C = 8
NCORES = 8
BSH = B // NCORES          # 4 samples per core
NPAIR = BSH * C            # 32 (b,c) pairs per core
NT = N // 128              # 16 node tiles
NG = 2                     # pair groups per core
GP = NPAIR // NG           # 16 pairs per group
GC = GP * L                # 384 moving columns per group
NSLAB = 4                  # transpose slabs per group (4 pairs each)
SP = GP // NSLAB           # pairs per slab
SW = SP * L                # 96 columns per slab

_CACHE = {}


def _build_bass():
    nc = bacc.Bacc(
        "TRN2", target_bir_lowering=False, debug=False, num_devices=NCORES)
    adjb = nc.declare_dram_parameter("adjb", [128, NT * N], FP8, isOutput=False)
    xsd = nc.declare_dram_parameter(
        "xs", [NG, NSLAB, 128, NT * 128], BF16, isOutput=False)
    mbd = nc.declare_dram_parameter("mb0", [NG, 128, NT * GC], BF16, isOutput=False)
    dinvd = nc.declare_dram_parameter("dinv", [128, NT], F32, isOutput=False)
    dinv4d = nc.declare_dram_parameter("dinv4", [128, NT], F32, isOutput=False)
    biasd = nc.declare_dram_parameter("biasN", [128, NT * GC], BF16, isOutput=False)
    wpd = nc.declare_dram_parameter("wp", [128, SP * 72], BF16, isOutput=False)
    od = nc.declare_dram_parameter("o", [NG, NT, 2, 128, GC], F32, isOutput=True)

    mult = mybir.AluOpType.mult
    add = mybir.AluOpType.add

    with tile.TileContext(nc) as tc:
        with (
            tc.tile_pool(name="cst", bufs=1) as cst,
            tc.tile_pool(name="adjp", bufs=1) as adjp,
            tc.tile_pool(name="mov", bufs=1) as mov,
            tc.tile_pool(name="ytp", bufs=1) as ytp,
            tc.tile_pool(name="rap", bufs=1) as rap,
            tc.tile_pool(name="wrk", bufs=1) as wrk,
            tc.tile_pool(name="otp", bufs=4) as otp,
            tc.tile_pool(name="psY", bufs=4, space="PSUM") as psY,
            tc.tile_pool(name="psS", bufs=2, space="PSUM") as psS,
            tc.tile_pool(name="psB", bufs=2, space="PSUM") as psB,
        ):
            def alloc_xs(g):
                tiles = []
                for s in range(NSLAB):
                    xsb = mov.tile([128, NT * 128], BF16, tag="xg", bufs=4,
                                   name="xsb")
                    nc.sync.dma_start(xsb[:], xsd[g, s])
                    tiles.append(xsb)
                return tiles

            # first group's x slabs go ahead of adj in the DMA stream so the
            # first y^T matmuls start as early as possible
            xs_first = alloc_xs(0)

            # per-u-tile DMAs so the first y^T matmuls can start while the
            # rest of adj is still in flight
            adj_sb = adjp.tile([128, NT * N], FP8)
            for u in range(NT):
                nc.sync.dma_start(
                    adj_sb[:, u * N:(u + 1) * N], adjb[:, u * N:(u + 1) * N])
            dinv_sb = cst.tile([128, NT], F32, tag="dinv")
            nc.sync.dma_start(dinv_sb[:], dinvd[:])
            dinv4_sb = cst.tile([128, NT], F32, tag="dinv4")
            nc.sync.dma_start(dinv4_sb[:], dinv4d[:])
            bias_sb = cst.tile([128, NT * GC], BF16, tag="biasN")
            nc.sync.dma_start(bias_sb[:], biasd[:])
            wp_sb = cst.tile([128, SP * 72], BF16, tag="wp")
            nc.sync.dma_start(wp_sb[:], wpd[:])

            def astile(u, vt):
                col = u * N + vt * 128
                return adj_sb[:, col:col + 128]

            for g in range(NG):
                mb = mov.tile([128, NT * GC], BF16, tag="mb")
                nc.sync.dma_start(mb[:], mbd[g])

                # Level Y, feature-major: y^T slab = x_slab^T-contracted with
                # adj as the MOVING operand. Stationary = x slab [128 nodes,
                # 128 cols] holding 4 pairs at 32-col offsets (l<24 used, rest
                # zero); out psum = y^T [4-pair rows, 512 node cols].
                yts = [
                    ytp.tile([128, NT * 128], BF16, tag=f"yt{s}",
                             name=f"yt{s}")
                    for s in range(NSLAB)
                ]
                # Smalls (emitted interleaved with y^T below):
                # [R1'|R4|R5](tile ut) = (y^T chunk)^T @ wp, node-major out.
                # One standard full-width matmul per (slab, ut): stationary =
                # the whole 4-pair y^T chunk, rhs = block-diagonal weight pack
                # (zeros route each pair's rows to its own 72-col output band)
                rall = rap.tile([128, NT, GP, 72], BF16, tag="rall")

                def smalls(s):
                    for ut in range(NT):
                        pm = psS.tile([128, 512], F32, name="pm", tag="pm")
                        nc.tensor.matmul(
                            pm[:, 0:SP * 72],
                            yts[s][:, ut * 128:(ut + 1) * 128],
                            wp_sb[:])
                        nc.vector.tensor_copy(
                            rall[:, ut, s * SP:(s + 1) * SP, :],
                            pm[:, 0:SP * 72])

                xsbs = xs_first if g == 0 else alloc_xs(g)
                for s in range(NSLAB):
                    xsb = xsbs[s]
                    pss = [
                        psY.tile([128, 512], F32, name="psy", tag="psy")
                        for cb in range(4)
                    ]
                    # u-outer: one ldweights per u feeds all 4 psum chunks,
                    # and the first block computes while later adj DMAs land
                    for u in range(NT):
                        for cb in range(4):
                            nc.tensor.matmul(
                                pss[cb][:], xsb[:, u * 128:(u + 1) * 128],
                                adj_sb[:, u * N + cb * 512:
                                       u * N + (cb + 1) * 512],
                                start=(u == 0), stop=(u == NT - 1))
                    for cb in range(4):
                        nc.vector.tensor_copy(
                            yts[s][:, cb * 512:(cb + 1) * 512], pss[cb][:])
                    if s >= 1:
                        smalls(s - 1)
                smalls(NSLAB - 1)

                # D5 = dinv * R5 (per source node)
                d5 = wrk.tile([128, NT * GC], BF16, tag="d5")
                for ut in range(NT):
                    nc.vector.tensor_scalar_mul(
                        d5[:, ut * GC:(ut + 1) * GC],
                        rall[:, ut, :, 48:72], dinv_sb[:, ut:ut + 1])

                # Level W: w = A^T R5; U2s = R4 + dinv*w
                u2s = wrk.tile([128, NT * GC], BF16, tag="u2s")
                for vt in range(NT):
                    ps = psB.tile([128, GC], F32)
                    for u in range(NT):
                        nc.tensor.matmul(
                            ps[:], astile(u, vt), rall[:, u, :, 48:72],
                            start=(u == 0), stop=(u == NT - 1))
                    nc.vector.scalar_tensor_tensor(
                        u2s[:, vt * GC:(vt + 1) * GC], ps[:],
                        dinv_sb[:, vt:vt + 1], rall[:, vt, :, 24:48],
                        op0=mult, op1=add)

                # Level T2: t = A^T D5; OUT0 = dinv4*(t + R4) + mb0
                for vt in range(NT):
                    ps = psB.tile([128, GC], F32)
                    for u in range(NT):
                        nc.tensor.matmul(
                            ps[:], astile(u, vt), d5[:, u * GC:(u + 1) * GC],
                            start=(u == 0), stop=(u == NT - 1))
                    t0 = otp.tile([128, GC], F32, tag="t0")
                    nc.vector.tensor_tensor(
                        t0[:], ps[:], rall[:, vt, :, 24:48], op=add)
                    t0b = otp.tile([128, GC], F32, tag="t0b")
                    nc.vector.scalar_tensor_tensor(
                        t0b[:], t0[:], dinv4_sb[:, vt:vt + 1],
                        mb[:, vt * GC:(vt + 1) * GC], op0=mult, op1=add)
                    nc.sync.dma_start(od[g, vt, 0], t0b[:])

                # Level A1: a1 = A^T U2s; OUT1 = dinv*a1 + R1' + biasN
                for vt in range(NT):
                    ps = psB.tile([128, GC], F32)
                    for u in range(NT):
                        nc.tensor.matmul(
                            ps[:], astile(u, vt), u2s[:, u * GC:(u + 1) * GC],
                            start=(u == 0), stop=(u == NT - 1))
                    t1 = otp.tile([128, GC], F32, tag="t1")
                    nc.vector.scalar_tensor_tensor(
                        t1[:], ps[:], dinv_sb[:, vt:vt + 1],
                        rall[:, vt, :, 0:24], op0=mult, op1=add)
                    t1b = otp.tile([128, GC], F32, tag="t1b")
                    nc.vector.tensor_tensor(
                        t1b[:], t1[:], bias_sb[:, vt * GC:(vt + 1) * GC], op=add)
                    nc.sync.dma_start(od[g, vt, 1], t1b[:])
    nc.compile()
    return nc


def _pack_moving(m):
    """[BSH, C, N, L] f32 -> [NG, 128, NT*GC] bf16 (pairs b-major)."""
    a = m.transpose(2, 0, 1, 3).reshape(NT, 128, NPAIR * L)
    a = a.reshape(NT, 128, NG, GC).transpose(2, 1, 0, 3).reshape(NG, 128, NT * GC)
    return np.ascontiguousarray(a).astype(ml_dtypes.bfloat16)


def kernel(x, adj, W_self, W_neigh, bias, _trace=False):
    x = np.asarray(x, dtype=np.float32)
    adj = np.asarray(adj, dtype=np.float32)
    W_self = np.asarray(W_self, dtype=np.float32)
    W_neigh = np.asarray(W_neigh, dtype=np.float32)
    bias = np.asarray(bias, dtype=np.float32)

    A00 = W_self[0].T @ W_self[1].T
    B01 = W_neigh[0].T @ W_self[1].T + W_self[0].T @ W_neigh[1].T
    C01 = W_neigh[0].T @ W_neigh[1].T
    indeg = adj.sum(0)
    deg = np.maximum(indeg, 1.0)
    s = (indeg >= 1).astype(np.float32)
    biasN = (bias[0] @ W_self[1].T + bias[1])[None, :] \
        + s[:, None] * (bias[0] @ W_neigh[1].T)[None, :]      # [N, L]

    adjb = np.ascontiguousarray(
        adj.reshape(NT, 128, N).transpose(1, 0, 2).reshape(128, NT * N)
    ).astype(ml_dtypes.float8_e4m3)
    dinv = np.ascontiguousarray((1.0 / deg).reshape(NT, 128).T).astype(np.float32)
    dinv4 = np.ascontiguousarray(4.0 * dinv)
    biasP = np.ascontiguousarray(
        np.broadcast_to(biasN.reshape(NT, 128, 1, L), (NT, 128, GP, L))
        .reshape(NT, 128, GC).transpose(1, 0, 2).reshape(128, NT * GC)
    ).astype(ml_dtypes.bfloat16)
    wp1 = np.concatenate([A00, B01, C01], axis=1)        # [24, 72]
    # block-diagonal: pair k's y^T rows (32k..32k+24) feed cols 72k..72k+72
    wp = np.zeros((128, SP * 72), dtype=np.float32)
    for k in range(SP):
        wp[32 * k:32 * k + L, 72 * k:72 * (k + 1)] = wp1
    wp = wp.astype(ml_dtypes.bfloat16)
    mb_all = 4.0 * (x @ A00) + biasN[None, None]

    if "nc" not in _CACHE:
        _CACHE["nc"] = _build_bass()
    nc = _CACHE["nc"]

    in_maps = []
    for c in range(NCORES):
        sl = slice(c * BSH, (c + 1) * BSH)
        # x slabs for the feature-major y^T matmul: [g, s, node_in_tile,
        # u*128 + 32*sp + l], zero-padded l=24..31
        pr = x[sl].reshape(NPAIR, NT, 128, L).transpose(0, 2, 1, 3)
        xs6 = np.zeros((NG, NSLAB, 128, NT, SP, 32), dtype=np.float32)
        for g in range(NG):
            for s_ in range(NSLAB):
                for sp in range(SP):
                    xs6[g, s_, :, :, sp, :L] = pr[g * GP + s_ * SP + sp]
        xs = np.ascontiguousarray(
            xs6.reshape(NG, NSLAB, 128, NT * 128)).astype(ml_dtypes.bfloat16)
        in_maps.append({
            "adjb": adjb,
            "xs": xs,
            "mb0": _pack_moving(mb_all[sl]),
            "dinv": dinv,
            "dinv4": dinv4,
            "biasN": biasP,
            "wp": wp,
        })

    res = run_bass_kernel_spmd(
        nc, in_maps, list(range(NCORES)), trace=_trace)

    out = np.empty((B, 2 * C, N, L), dtype=np.float32)
    for c in range(NCORES):
        o = np.asarray(res.results[c]["o"], dtype=np.float32)
        # [NG, NT, 2, 128, GC] -> (g, vt, k, p, pin, l)
        a = o.reshape(NG, NT, 2, 128, GP, L)
        # pairs = g*GP + pin, b-major: b_local = pairs//C, ch = pairs%C
        a = a.transpose(0, 4, 2, 1, 3, 5).reshape(NPAIR, 2, N, L)
        a = a.reshape(BSH, C, 2, N, L).reshape(BSH, 2 * C, N, L)
        out[c * BSH:(c + 1) * BSH] = a
    if _trace:
        return out, res
    return out


# revision 43
# speedup vs baseline: 1.1661x; 1.1661x over previous
"""GraphSAGE (2-layer, DGL SAGEConv-mean) Trainium2 kernel — y-scheme.

Data-parallel over B (4 samples per core, 8 cores). Per (b,c) pair, with
A=adj, deg=max(indeg,1), D=diag(deg):

  y  = A^T x                      (level Y, 24 cols/pair)
  [R1'|R4|R5] = y @ [A00|B01|C01] (PE transpose of y + small matmuls,
                                   output lands node-major directly)
  t  = A^T (D^{-1} R5)            (level T2)
  w  = A^T R5                     (level W)
  OUT0 = dinv4*(t + R4) + (4*x@A00 + biasN)        [host-folded mb0]
  OUT1 = dinv*(A^T (R4 + D^{-1} w)) + R1' + biasN  (level A1)

vs the previous 6-level scheme this applies A^T to 4 slabs per pair
instead of 6 (96 vs 144 moving cols/pair). adj is stored fp8_e4m3
(exact for 0/1), halving its SBUF/DMA footprint. Small-weight products
use lhsT = y^T chunks so results come out node-major (no back-transpose).
"""
import sys

sys.path.insert(0, "/opt/trn_rl_repo")

import numpy as np
import ml_dtypes

from concourse import bass, bacc, tile, mybir
from concourse.bass_utils import run_bass_kernel_spmd

BF16 = mybir.dt.bfloat16
F32 = mybir.dt.float32
FP8 = mybir.dt.float8e4

N = 2048
L = 24
B = 32
C = 8
NCORES = 8
BSH = B // NCORES          # 4 samples per core
NPAIR = BSH * C            # 32 (b,c) pairs per core
NT = N // 128              # 16 node tiles
NG = 2                     # pair groups per core
GP = NPAIR // NG           # 16 pairs per group
GC = GP * L                # 384 moving columns per group
NSLAB = 4                  # transpose slabs per group (4 pairs each)
SP = GP // NSLAB           # pairs per slab
SW = SP * L                # 96 columns per slab

_CACHE = {}


def _build_bass():
    nc = bacc.Bacc(
        "TRN2", target_bir_lowering=False, debug=False, num_devices=NCORES)
    adjb = nc.declare_dram_parameter("adjb", [128, NT * N], BF16, isOutput=False)
    xsd = nc.declare_dram_parameter(
        "xs", [NG, NSLAB, 128, NT * 128], BF16, isOutput=False)
    mbd = nc.declare_dram_parameter("mb0", [NG, 128, NT * GC], BF16, isOutput=False)
    dinvd = nc.declare_dram_parameter("dinv", [128, NT], F32, isOutput=False)
    dinv4d = nc.declare_dram_parameter("dinv4", [128, NT], F32, isOutput=False)
    biasd = nc.declare_dram_parameter("biasN", [128, NT * GC], BF16, isOutput=False)
    wpd = nc.declare_dram_parameter("wp", [128, SP * 72], BF16, isOutput=False)
    od = nc.declare_dram_parameter("o", [NG, NT, 2, 128, GC], F32, isOutput=True)

    mult = mybir.AluOpType.mult
    add = mybir.AluOpType.add

    with tile.TileContext(nc) as tc:
        with (
            tc.tile_pool(name="cst", bufs=1) as cst,
            tc.tile_pool(name="adjp", bufs=1) as adjp,
            tc.tile_pool(name="mov", bufs=1) as mov,
            tc.tile_pool(name="ytp", bufs=1) as ytp,
            tc.tile_pool(name="rap", bufs=1) as rap,
            tc.tile_pool(name="wrk", bufs=1) as wrk,
            tc.tile_pool(name="otp", bufs=4) as otp,
            tc.tile_pool(name="psY", bufs=4, space="PSUM") as psY,
            tc.tile_pool(name="psS", bufs=2, space="PSUM") as psS,
            tc.tile_pool(name="psB", bufs=2, space="PSUM") as psB,
        ):
            def alloc_xs(g):
                tiles = []
                for s in range(NSLAB):
                    xsb = mov.tile([128, NT * 128], BF16, tag="xg", bufs=4,
                                   name="xsb")
                    nc.sync.dma_start(xsb[:], xsd[g, s])
                    tiles.append(xsb)
                return tiles

            # first group's x slabs go ahead of adj in the DMA stream so the
            # first y^T matmuls start as early as possible
            xs_first = alloc_xs(0)

            # per-u-tile DMAs so the first y^T matmuls can start while the
            # rest of adj is still in flight
            adj_sb = adjp.tile([128, NT * N], BF16)
            for u in range(NT):
                nc.sync.dma_start(
                    adj_sb[:, u * N:(u + 1) * N], adjb[:, u * N:(u + 1) * N])
            dinv_sb = cst.tile([128, NT], F32, tag="dinv")
            nc.sync.dma_start(dinv_sb[:], dinvd[:])
            dinv4_sb = cst.tile([128, NT], F32, tag="dinv4")
            nc.sync.dma_start(dinv4_sb[:], dinv4d[:])
            bias_sb = cst.tile([128, NT * GC], BF16, tag="biasN")
            nc.sync.dma_start(bias_sb[:], biasd[:])
            wp_sb = cst.tile([128, SP * 72], BF16, tag="wp")
            nc.sync.dma_start(wp_sb[:], wpd[:])

            def astile(u, vt):
                col = u * N + vt * 128
                return adj_sb[:, col:col + 128]

            for g in range(NG):
                mb = mov.tile([128, NT * GC], BF16, tag="mb")
                nc.sync.dma_start(mb[:], mbd[g])

                # Level Y, feature-major: y^T slab = x_slab^T-contracted with
                # adj as the MOVING operand. Stationary = x slab [128 nodes,
                # 128 cols] holding 4 pairs at 32-col offsets (l<24 used, rest
                # zero); out psum = y^T [4-pair rows, 512 node cols].
                yts = [
                    ytp.tile([128, NT * 128], BF16, tag=f"yt{s}",
                             name=f"yt{s}")
                    for s in range(NSLAB)
                ]
                # Smalls (emitted interleaved with y^T below):
                # [R1'|R4|R5](tile ut) = (y^T chunk)^T @ wp, node-major out.
                # One standard full-width matmul per (slab, ut): stationary =
                # the whole 4-pair y^T chunk, rhs = block-diagonal weight pack
                # (zeros route each pair's rows to its own 72-col output band)
                rall = rap.tile([128, NT, GP, 72], BF16, tag="rall")

                def smalls(s):
                    for ut in range(NT):
                        pm = psS.tile([128, 512], F32, name="pm", tag="pm")
                        nc.tensor.matmul(
                            pm[:, 0:SP * 72],
                            yts[s][:, ut * 128:(ut + 1) * 128],
                            wp_sb[:])
                        nc.vector.tensor_copy(
                            rall[:, ut, s * SP:(s + 1) * SP, :],
                            pm[:, 0:SP * 72])

                xsbs = xs_first if g == 0 else alloc_xs(g)
                for s in range(NSLAB):
                    xsb = xsbs[s]
                    pss = [
                        psY.tile([128, 512], F32, name="psy", tag="psy")
                        for cb in range(4)
                    ]
                    # u-outer: one ldweights per u feeds all 4 psum chunks,
                    # and the first block computes while later adj DMAs land
                    for u in range(NT):
                        for cb in range(4):
                            nc.tensor.matmul(
                                pss[cb][:], xsb[:, u * 128:(u + 1) * 128],
                                adj_sb[:, u * N + cb * 512:
                                       u * N + (cb + 1) * 512],
                                start=(u == 0), stop=(u == NT - 1))
                    for cb in range(4):
                        nc.vector.tensor_copy(
                            yts[s][:, cb * 512:(cb + 1) * 512], pss[cb][:])
                    if s >= 1:
                        smalls(s - 1)
                smalls(NSLAB - 1)

                # D5 = dinv * R5 (per source node)
                d5 = wrk.tile([128, NT * GC], BF16, tag="d5")
                for ut in range(NT):
                    nc.vector.tensor_scalar_mul(
                        d5[:, ut * GC:(ut + 1) * GC],
                        rall[:, ut, :, 48:72], dinv_sb[:, ut:ut + 1])

                # Level W: w = A^T R5; U2s = R4 + dinv*w
                u2s = wrk.tile([128, NT * GC], BF16, tag="u2s")
                for vt in range(NT):
                    ps = psB.tile([128, GC], F32)
                    for u in range(NT):
                        nc.tensor.matmul(
                            ps[:], astile(u, vt), rall[:, u, :, 48:72],
                            start=(u == 0), stop=(u == NT - 1))
                    nc.vector.scalar_tensor_tensor(
                        u2s[:, vt * GC:(vt + 1) * GC], ps[:],
                        dinv_sb[:, vt:vt + 1], rall[:, vt, :, 24:48],
                        op0=mult, op1=add)

                # Level T2: t = A^T D5; OUT0 = dinv4*(t + R4) + mb0
                for vt in range(NT):
                    ps = psB.tile([128, GC], F32)
                    for u in range(NT):
                        nc.tensor.matmul(
                            ps[:], astile(u, vt), d5[:, u * GC:(u + 1) * GC],
                            start=(u == 0), stop=(u == NT - 1))
                    t0 = otp.tile([128, GC], F32, tag="t0")
                    nc.vector.tensor_tensor(
                        t0[:], ps[:], rall[:, vt, :, 24:48], op=add)
                    t0b = otp.tile([128, GC], F32, tag="t0b")
                    nc.vector.scalar_tensor_tensor(
                        t0b[:], t0[:], dinv4_sb[:, vt:vt + 1],
                        mb[:, vt * GC:(vt + 1) * GC], op0=mult, op1=add)
                    nc.sync.dma_start(od[g, vt, 0], t0b[:])

                # Level A1: a1 = A^T U2s; OUT1 = dinv*a1 + R1' + biasN
                for vt in range(NT):
                    ps = psB.tile([128, GC], F32)
                    for u in range(NT):
                        nc.tensor.matmul(
                            ps[:], astile(u, vt), u2s[:, u * GC:(u + 1) * GC],
                            start=(u == 0), stop=(u == NT - 1))
                    t1 = otp.tile([128, GC], F32, tag="t1")
                    nc.vector.scalar_tensor_tensor(
                        t1[:], ps[:], dinv_sb[:, vt:vt + 1],
                        rall[:, vt, :, 0:24], op0=mult, op1=add)
                    t1b = otp.tile([128, GC], F32, tag="t1b")
                    nc.vector.tensor_tensor(
                        t1b[:], t1[:], bias_sb[:, vt * GC:(vt + 1) * GC], op=add)
                    nc.sync.dma_start(od[g, vt, 1], t1b[:])
    nc.compile()
    return nc


def _pack_moving(m):
    """[BSH, C, N, L] f32 -> [NG, 128, NT*GC] bf16 (pairs b-major)."""
    a = m.transpose(2, 0, 1, 3).reshape(NT, 128, NPAIR * L)
    a = a.reshape(NT, 128, NG, GC).transpose(2, 1, 0, 3).reshape(NG, 128, NT * GC)
    return np.ascontiguousarray(a).astype(ml_dtypes.bfloat16)


def kernel(x, adj, W_self, W_neigh, bias, _trace=False):
    x = np.asarray(x, dtype=np.float32)
    adj = np.asarray(adj, dtype=np.float32)
    W_self = np.asarray(W_self, dtype=np.float32)
    W_neigh = np.asarray(W_neigh, dtype=np.float32)
    bias = np.asarray(bias, dtype=np.float32)

    A00 = W_self[0].T @ W_self[1].T
    B01 = W_neigh[0].T @ W_self[1].T + W_self[0].T @ W_neigh[1].T
    C01 = W_neigh[0].T @ W_neigh[1].T
    indeg = adj.sum(0)
    deg = np.maximum(indeg, 1.0)
    s = (indeg >= 1).astype(np.float32)
    biasN = (bias[0] @ W_self[1].T + bias[1])[None, :] \
        + s[:, None] * (bias[0] @ W_neigh[1].T)[None, :]      # [N, L]

    adjb = np.ascontiguousarray(
        adj.reshape(NT, 128, N).transpose(1, 0, 2).reshape(128, NT * N)
    ).astype(ml_dtypes.bfloat16)
    dinv = np.ascontiguousarray((1.0 / deg).reshape(NT, 128).T).astype(np.float32)
    dinv4 = np.ascontiguousarray(4.0 * dinv)
    biasP = np.ascontiguousarray(
        np.broadcast_to(biasN.reshape(NT, 128, 1, L), (NT, 128, GP, L))
        .reshape(NT, 128, GC).transpose(1, 0, 2).reshape(128, NT * GC)
    ).astype(ml_dtypes.bfloat16)
    wp1 = np.concatenate([A00, B01, C01], axis=1)        # [24, 72]
    # block-diagonal: pair k's y^T rows (32k..32k+24) feed cols 72k..72k+72
    wp = np.zeros((128, SP * 72), dtype=np.float32)
    for k in range(SP):
        wp[32 * k:32 * k + L, 72 * k:72 * (k + 1)] = wp1
    wp = wp.astype(ml_dtypes.bfloat16)
    mb_all = 4.0 * (x @ A00) + biasN[None, None]

    if "nc" not in _CACHE:
        _CACHE["nc"] = _build_bass()
    nc = _CACHE["nc"]

    in_maps = []
    for c in range(NCORES):
        sl = slice(c * BSH, (c + 1) * BSH)
        # x slabs for the feature-major y^T matmul: [g, s, node_in_tile,
        # u*128 + 32*sp + l], zero-padded l=24..31
        pr = x[sl].reshape(NPAIR, NT, 128, L).transpose(0, 2, 1, 3)
        xs6 = np.zeros((NG, NSLAB, 128, NT, SP, 32), dtype=np.float32)
        for g in range(NG):
            for s_ in range(NSLAB):
                for sp in range(SP):
                    xs6[g, s_, :, :, sp, :L] = pr[g * GP + s_ * SP + sp]
        xs = np.ascontiguousarray(
            xs6.reshape(NG, NSLAB, 128, NT * 128)).astype(ml_dtypes.bfloat16)
        in_maps.append({
            "adjb": adjb,
            "xs": xs,
            "mb0": _pack_moving(mb_all[sl]),
            "dinv": dinv,
            "dinv4": dinv4,
            "biasN": biasP,
            "wp": wp,
        })

    res = run_bass_kernel_spmd(
        nc, in_maps, list(range(NCORES)), trace=_trace)

    out = np.empty((B, 2 * C, N, L), dtype=np.float32)
    for c in range(NCORES):
        o = np.asarray(res.results[c]["o"], dtype=np.float32)
        # [NG, NT, 2, 128, GC] -> (g, vt, k, p, pin, l)
        a = o.reshape(NG, NT, 2, 128, GP, L)
        # pairs = g*GP + pin, b-major: b_local = pairs//C, ch = pairs%C
        a = a.transpose(0, 4, 2, 1, 3, 5).reshape(NPAIR, 2, N, L)
        a = a.reshape(BSH, C, 2, N, L).reshape(BSH, 2 * C, N, L)
        out[c * BSH:(c + 1) * BSH] = a
    if _trace:
        return out, res
    return out
